# revision 28
# baseline (speedup 1.0000x reference)
"""AttentionDTI forward pass on 8 Trainium2 NeuronCores (pure data parallel).

Batch of 8 peptide/MHC pairs; one batch element per core, weights
replicated. The 4D additive-attention tensor h[b,p,m,c] =
relu(p_att + m_att) is never materialized in HBM: since the following
linear layer is, well, linear, mean_m(h @ Wa) == mean_m(h) @ Wa, so the
kernel only accumulates hp[c,p] = sum_m h and hm[c,m] = sum_p h on the
fly. hp comes from fused relu+bias+accum; hm is accumulated in PSUM by
streaming h tiles through the TensorEngine against a stationary
identity matrix.

Matmul dtype strategy: float32r everywhere the moving dim is large
(1 cycle/row vs fp32's 4, ~6e-5 input rounding error), bf16 on the
h-tile path. FP32r ISA restrictions: both operands f32r-typed memory,
EVEN moving-column count, even dst counts + 8B-aligned dst offset +
dst start partition 0 — hence all conv/proj extents are padded to even
lengths (pad columns hold finite garbage, excluded from reductions).
FC head streams the weight matrices as the moving operand against
single-column stationary activations; the resulting [1, N] rows fold
back to partition-column layout with PE outer-product transposes
(row-chunk x ones -> psum column; W in natural 128-row blocks).

The 4D-attention h tiles are produced by Scalar (fused
relu+bias+accum_out activation) and DVE (tensor_scalar relu + a second
tensor_scalar whose op1 acts as the accumulate-reduce) in parallel;
some pairs are summed on DVE before the identity matmul, the rest feed
the PE unpaired — tuned so Scalar/DVE/PE all run near-saturated.

Environment constraints discovered empirically (this axon terminal):
  - GPSIMD/Pool ucode ops (SWDGE DMA, gpsimd memset/iota) hang: all DMAs
    go through the sync-engine HWDGE, memsets through the VectorEngine.
  - scalar_tensor_tensor hangs: only tensor_scalar / tensor_tensor /
    tensor_reduce / activation / matmul are used.
  - walrus here allows at most ONE semaphore wait per instruction:
    _split_excess_waits() rewrites the Tile-scheduled program, moving
    excess waits onto standalone InstEventSemaphore instructions.
"""
import sys

_BASS_ROOT = '/opt/trn_rl_repo'
if _BASS_ROOT not in sys.path:
    sys.path.insert(0, _BASS_ROOT)

import numpy as np
import ml_dtypes

import concourse.bass as bass
import concourse.tile as tile
from concourse import mybir
from concourse.bass_utils import run_bass_kernel_spmd

F32 = mybir.dt.float32
F32R = mybir.dt.float32r
BF16 = mybir.dt.bfloat16
ALU = mybir.AluOpType
AF = mybir.ActivationFunctionType
AX = mybir.AxisListType

# model dims (hardcoded from the problem spec)
B = 8
LP, LM, DIM, CONV = 100, 1000, 64, 40
C2, C4 = CONV * 2, CONV * 4          # 80, 160
K1, K2, K3 = 4, 6, 8
LP1, LP2, LP3 = 97, 92, 85           # peptide conv output lengths (valid)
LM1, LM2, LM3 = 997, 992, 985        # MHC conv output lengths (valid)
# even-padded extents for fp32r matmuls (reads stay in range; pad cols finite)
PEE, LP1E, LP2E, LP3E = 104, 100, 94, 86
MEE, LM1E, LM2E, LM3E = 1004, 1000, 994, 986
MPAD = 992                           # LM3 padded to a multiple of 32
NEG = -30000.0                       # -inf stand-in that survives bf16
NPB = 22                             # ceil(85/4) packed p-groups for the c[128:160] chunk

_ctr = [0]
DEBUG = False


def _split_excess_waits(nc, max_waits=1):
    n_split = 0
    for f in nc.m.functions:
        for b in f.blocks:
            insts = list(b.instructions)
            out = []
            changed = False
            for inst in insts:
                si = inst.sync_info
                waits = list(si.on_wait) if (si is not None and si.on_wait) else []
                if len(waits) > max_waits:
                    changed = True
                    n_split += 1
                    keep = max(1, max_waits)
                    head, tail = waits[:-keep], waits[-keep:]
                    for i in range(0, len(head), keep):
                        chunk = head[i:i + keep]
                        nop = mybir.InstEventSemaphore(
                            name=f"ant-wait-split-{_ctr[0]}", ins=[], outs=[])
                        _ctr[0] += 1
                        nop.engine = inst.engine
                        nop.sync_info = mybir.SyncInfo(on_wait=chunk, on_update=[])
                        nc.register_instruction(nop)
                        out.append(nop)
                    upd = list(si.on_update) if si.on_update else []
                    inst.sync_info = mybir.SyncInfo(on_wait=tail, on_update=upd)
                out.append(inst)
            if changed:
                b.instructions = out
    return n_split


def _conv_matmuls(nc, psum, wtile, x, k_taps, co_lo, co_hi, m_lo, m_hi, cout_stride):
    """Accumulate a valid 1-D conv as k shifted matmuls into `psum`.

    psum: [co_hi-co_lo, m_hi-m_lo]; wtile: [ci, K*cout_stride] with tap k
    at columns [k*cout_stride, (k+1)*cout_stride); x: [ci, L].
    """
    for k in range(k_taps):
        nc.tensor.matmul(
            psum,
            wtile[:, k * cout_stride + co_lo: k * cout_stride + co_hi],
            x[:, m_lo + k: m_hi + k],
            start=(k == 0), stop=(k == k_taps - 1))


def _build_program():
    nc = bass.Bass("TRN2", target_bir_lowering=False, debug=False)

    def par(name, shape, dtype=F32):
        return nc.declare_dram_parameter(name, list(shape), dtype, isOutput=False)

    # merged params: one DMA per group, ordered by first use
    # pe/me are host-gathered embeddings, padded and 2-tap stacked:
    # rows 0:64 = emb[:, j], rows 64:128 = emb[:, j+1]
    pe_e = par("pe_st", [128, PEE], F32R)
    me_e = par("me_st", [128, MEE], F32R)
    cw1_e = par("cw1", [128, 4 * CONV], F32R)        # stacked tap-pairs pep|mhc
    bias_e = par("biases", [128, 14])                # all conv/proj biases
    cw2_e = par("cw2", [CONV, 2 * K2 * C2], F32R)    # pw2 | mw2
    cw3_e = par("cw3", [C2, 2 * K3 * C4], F32R)      # pw3 | mw3
    watt_e = par("watt", [128, 4 * C4], F32R)        # Wpa|Wma|Wa/985|Wa/85 rows 0:128
    wattb_e = par("wattb", [32, 4 * C4], F32R)       # same, rows 128:160
    ids_e = par("ids", [128, 160], BF16)             # id128 | idst
    # FC weights, blocked for weight-as-moving-operand matmuls
    w1a_e = par("w1a", [128, 2048], F32R)            # W1 rows 0:128 | 160:288
    w1b_e = par("w1b", [32, 2048], F32R)             # W1 rows 128:160 | 288:320
    w2_e = par("w2", [128, 8 * 1024], F32R)
    w3_e = par("w3", [128, 8 * 512], F32R)
    wo_e = par("wo", [128, 8], F32R)
    fcb_e = par("fcb", [128, 20])                    # b1 | b2 | b3
    bo_e = par("bo", [1, 2])

    out_e = nc.declare_dram_parameter("out", [1, 2], F32, isOutput=True)

    with tile.TileContext(nc) as tc:
        with tc.tile_pool(name="consts", bufs=1) as cp, \
             tc.tile_pool(name="work", bufs=1) as wp, \
             tc.tile_pool(name="hpool", bufs=12) as hpool, \
             tc.tile_pool(name="ps_hm", bufs=1, space="PSUM") as ps_hm, \
             tc.tile_pool(name="ps_work", bufs=2, space="PSUM") as ps:

            def load(ext, shape, dtype=F32, name=None):
                t = cp.tile(shape, dtype, name=name or ext.name + "_sb")
                nc.sync.dma_start(out=t, in_=ext[:])
                return t

            # ---- merged constant loads, in first-use order ----
            pe = load(pe_e, [128, PEE], F32R)
            cw1 = load(cw1_e, [128, 4 * CONV], F32R)
            biases = load(bias_e, [128, 14])
            me = load(me_e, [128, MEE], F32R)
            cw2 = load(cw2_e, [CONV, 2 * K2 * C2], F32R)
            cw3 = load(cw3_e, [C2, 2 * K3 * C4], F32R)
            watt = load(watt_e, [128, 4 * C4], F32R)
            wattb = load(wattb_e, [32, 4 * C4], F32R)
            ids = load(ids_e, [128, 160], BF16)
            # views into the merged tiles
            pw1 = cw1[:, 0:2 * CONV]; mw1 = cw1[:, 2 * CONV:4 * CONV]
            pw2 = cw2[:, 0:K2 * C2]; mw2 = cw2[:, K2 * C2:2 * K2 * C2]
            pw3 = cw3[:, 0:K3 * C4]; mw3 = cw3[:, K3 * C4:2 * K3 * C4]
            pb1 = biases[0:CONV, 0:1]; pb2 = biases[0:C2, 1:2]; pb3 = biases[:, 2:4]
            mb1 = biases[0:CONV, 4:5]; mb2 = biases[0:C2, 5:6]; mb3 = biases[:, 6:8]
            bpa = biases[:, 8:10]; bma = biases[:, 10:12]; ba = biases[:, 12:14]
            wpaa = watt[:, 0:C4]; wmaa = watt[:, C4:2 * C4]
            wcaa = watt[:, 2 * C4:3 * C4]; wmaa2 = watt[:, 3 * C4:4 * C4]
            wpab = wattb[:, 0:C4]; wmab = wattb[:, C4:2 * C4]
            wcab = wattb[:, 2 * C4:3 * C4]; wmab2 = wattb[:, 3 * C4:4 * C4]
            id128 = ids[:, 0:128]; idst = ids[:, 128:160]


            # ---- peptide conv stack (even-padded extents) ----
            # conv1 as 2 stacked tap-pair matmuls (contraction 128)
            px1_ps = ps.tile([CONV, LP1E], F32, name="px1_ps", tag="ps")
            for k2 in range(2):
                nc.tensor.matmul(px1_ps, pw1[:, k2 * CONV:(k2 + 1) * CONV],
                                 pe[:, 2 * k2:2 * k2 + LP1E],
                                 start=(k2 == 0), stop=(k2 == 1))
            px1 = wp.tile([CONV, LP1E], F32R, name="px1")
            nc.scalar.activation(out=px1, in_=px1_ps, func=AF.Relu, bias=pb1[:, 0:1])

            px2_ps = ps.tile([C2, LP2E], F32, name="px2_ps", tag="ps")
            _conv_matmuls(nc, px2_ps, pw2, px1, K2, 0, C2, 0, LP2E, C2)
            px2 = wp.tile([C2, LP2E], F32R, name="px2")
            nc.scalar.activation(out=px2, in_=px2_ps, func=AF.Relu, bias=pb2[:, 0:1])

            pc0_ps = ps.tile([128, LP3E], F32, name="pc0_ps", tag="ps")
            _conv_matmuls(nc, pc0_ps, pw3, px2, K3, 0, 128, 0, LP3E, C4)
            pc0 = wp.tile([128, LP3E], F32R, name="pc0")
            nc.scalar.activation(out=pc0, in_=pc0_ps, func=AF.Relu, bias=pb3[:, 0:1])
            pc1_ps = ps.tile([32, LP3E], F32, name="pc1_ps", tag="ps")
            _conv_matmuls(nc, pc1_ps, pw3, px2, K3, 128, C4, 0, LP3E, C4)
            pc1 = wp.tile([32, LP3E], F32R, name="pc1")
            nc.scalar.activation(out=pc1, in_=pc1_ps, func=AF.Relu, bias=pb3[0:32, 1:2])

            # ---- MHC conv stack (free dim chunked to <=512, even extents) ----
            mx1_ps = ps.tile([CONV, LM1E], F32, name="mx1_ps", tag="ps")
            for lo, hi in ((0, 512), (512, LM1E)):
                for k2 in range(2):
                    nc.tensor.matmul(mx1_ps[:, lo:hi], mw1[:, k2 * CONV:(k2 + 1) * CONV],
                                     me[:, lo + 2 * k2:hi + 2 * k2],
                                     start=(k2 == 0), stop=(k2 == 1))
            mx1 = wp.tile([CONV, LM1E], F32R, name="mx1")
            nc.scalar.activation(out=mx1, in_=mx1_ps, func=AF.Relu, bias=mb1[:, 0:1])

            mx2_ps = ps.tile([C2, LM2E], F32, name="mx2_ps", tag="ps")
            _conv_matmuls(nc, mx2_ps[:, 0:512], mw2, mx1, K2, 0, C2, 0, 512, C2)
            _conv_matmuls(nc, mx2_ps[:, 512:LM2E], mw2, mx1, K2, 0, C2, 512, LM2E, C2)
            mx2 = wp.tile([C2, LM2E], F32R, name="mx2")
            nc.scalar.activation(out=mx2, in_=mx2_ps, func=AF.Relu, bias=mb2[:, 0:1])

            mc0_ps = ps.tile([128, LM3E], F32, name="mc0_ps", tag="ps")
            _conv_matmuls(nc, mc0_ps[:, 0:512], mw3, mx2, K3, 0, 128, 0, 512, C4)
            _conv_matmuls(nc, mc0_ps[:, 512:LM3E], mw3, mx2, K3, 0, 128, 512, LM3E, C4)
            mc0 = wp.tile([128, LM3E], F32R, name="mc0")
            nc.scalar.activation(out=mc0, in_=mc0_ps, func=AF.Relu, bias=mb3[:, 0:1])
            mc1_ps = ps.tile([32, LM3E], F32, name="mc1_ps", tag="ps")
            _conv_matmuls(nc, mc1_ps[:, 0:512], mw3, mx2, K3, 128, C4, 0, 512, C4)
            _conv_matmuls(nc, mc1_ps[:, 512:LM3E], mw3, mx2, K3, 128, C4, 512, LM3E, C4)
            mc1 = wp.tile([32, LM3E], F32R, name="mc1")
            nc.scalar.activation(out=mc1, in_=mc1_ps, func=AF.Relu, bias=mb3[0:32, 1:2])

            # ---- attention projections ----
            # pa[c,p] = sum_c' pc[c',p] * Wpa[c',c] + bpa[c]
            pa0_ps = ps.tile([128, LP3E], F32, name="pa0_ps", tag="ps")
            nc.tensor.matmul(pa0_ps, wpaa[:, 0:128], pc0, start=True, stop=False)
            nc.tensor.matmul(pa0_ps, wpab[:, 0:128], pc1, start=False, stop=True)
            pa0 = wp.tile([128, LP3E], F32, name="pa0")
            nc.scalar.add(pa0, pa0_ps, bpa[:, 0:1])

            pa1_ps = ps.tile([32, LP3E], F32, name="pa1_ps", tag="ps")
            nc.tensor.matmul(pa1_ps, wpaa[:, 128:C4], pc0, start=True, stop=False)
            nc.tensor.matmul(pa1_ps, wpab[:, 128:C4], pc1, start=False, stop=True)
            # pack 4 p-positions per 32-row block straight from PSUM:
            # pa1p[32j+d, g] = pa1_ps[d, 4g+j] + bpa (strided source, no DMA)
            pa1p = wp.tile([128, NPB], F32, name="pa1p")
            nc.vector.memset(pa1p, NEG)
            pa1_g = pa1_ps[:, 0:84].rearrange("d (g f) -> d g f", f=4)
            for j in range(4):
                nc.scalar.add(pa1p[32 * j:32 * j + 32, 0:NPB - 1],
                              pa1_g[:, :, j], bpa[0:32, 1:2])
            nc.scalar.add(pa1p[0:32, NPB - 1:NPB], pa1_ps[:, 84:85], bpa[0:32, 1:2])

            # ma[c,m] = sum_c' mc[c',m] * Wma[c',c] + bma[c]  (bf16, m padded with NEG)
            ma0_ps = ps.tile([128, LM3E], F32, name="ma0_ps", tag="ps")
            nc.tensor.matmul(ma0_ps[:, 0:512], wmaa[:, 0:128], mc0[:, 0:512], start=True, stop=False)
            nc.tensor.matmul(ma0_ps[:, 0:512], wmab[:, 0:128], mc1[:, 0:512], start=False, stop=True)
            nc.tensor.matmul(ma0_ps[:, 512:LM3E], wmaa[:, 0:128], mc0[:, 512:LM3E], start=True, stop=False)
            nc.tensor.matmul(ma0_ps[:, 512:LM3E], wmab[:, 0:128], mc1[:, 512:LM3E], start=False, stop=True)
            ma0 = wp.tile([128, MPAD], BF16, name="ma0")
            nc.vector.memset(ma0, NEG)
            nc.scalar.add(ma0[:, 0:LM3], ma0_ps[:, 0:LM3], bma[:, 0:1])

            ma1_ps = ps.tile([32, LM3E], F32, name="ma1_ps", tag="ps")
            nc.tensor.matmul(ma1_ps[:, 0:512], wmaa[:, 128:C4], mc0[:, 0:512], start=True, stop=False)
            nc.tensor.matmul(ma1_ps[:, 0:512], wmab[:, 128:C4], mc1[:, 0:512], start=False, stop=True)
            nc.tensor.matmul(ma1_ps[:, 512:LM3E], wmaa[:, 128:C4], mc0[:, 512:LM3E], start=True, stop=False)
            nc.tensor.matmul(ma1_ps[:, 512:LM3E], wmab[:, 128:C4], mc1[:, 512:LM3E], start=False, stop=True)
            # replicate 4x vertically for the packed c[128:160] loop, writing
            # each 32-row block straight from PSUM (no DMA on the critical path)
            ma1p = wp.tile([128, MPAD], BF16, name="ma1p")
            nc.vector.memset(ma1p, NEG)
            for j in range(4):
                nc.scalar.add(ma1p[32 * j:32 * (j + 1), 0:LM3],
                              ma1_ps[:, 0:LM3], bma[0:32, 1:2])

            # FC weights: emitted here so their (large) DMAs queue behind the
            # conv-phase pack DMAs and stream in during the attention phase
            w1a = load(w1a_e, [128, 2048], F32R)
            w1b = load(w1b_e, [32, 2048], F32R)
            w2 = load(w2_e, [128, 8192], F32R)
            w3 = load(w3_e, [128, 4096], F32R)
            wo = load(wo_e, [128, 8], F32R)
            fcbias = load(fcb_e, [128, 20])
            bo = load(bo_e, [1, 2])
            w1r0 = w1a[:, 0:1024]; w1r2 = w1a[:, 1024:2048]
            w1r1 = w1b[:, 0:1024]; w1r3 = w1b[:, 1024:2048]
            b1 = fcbias[:, 0:8]; b2 = fcbias[:, 8:16]; b3 = fcbias[:, 16:20]

            # ---- 4D attention reductions ----
            hp0 = wp.tile([128, 88], F32R, name="hp0")
            hp1p = wp.tile([128, NPB], F32R, name="hp1p")
            hm0_ps = ps_hm.tile([128, MPAD], F32, name="hm0_ps")
            hm1_ps = ps_hm.tile([32, MPAD], F32, name="hm1_ps")

            # h tiles are produced in pairs — Scalar (fused relu+bias+accum
            # activation) and DVE (tensor_scalar add-bias/max-0 with accum) in
            # parallel — then pair-summed on DVE so the TensorEngine only sees
            # half as many identity matmuls.
            with nc.allow_low_precision(reason="f32r accum is 4-byte fp32 bits"):
                def produce_scalar(src, bias, accu):
                    h = hpool.tile([128, MPAD], BF16, tag="h", name="h")
                    nc.scalar.activation(out=h, in_=src, func=AF.Relu,
                                         bias=bias, accum_out=accu)
                    return h

                dve_scr = wp.tile([128, MPAD], BF16, name="dve_scr")

                def produce_dve(src, bias, accu):
                    h = hpool.tile([128, MPAD], BF16, tag="h", name="h")
                    nc.vector.tensor_scalar(out=h, in0=src, scalar1=bias,
                                            scalar2=0.0, op0=ALU.add, op1=ALU.max)
                    # second ts: out is a dummy copy, op1 acts as the accum
                    # reduce op -> accu = sum_m h
                    nc.vector.tensor_scalar(out=dve_scr, in0=h, scalar1=0.0,
                                            scalar2=None, op0=ALU.add, op1=ALU.add,
                                            accum_out=accu)
                    return h

                jobs = [(ma0, pa0[:, p:p + 1], hp0[:, p:p + 1], hm0_ps, id128, 0)
                        for p in range(LP3)]
                jobs += [(ma1p, pa1p[:, g:g + 1], hp1p[:, g:g + 1], hm1_ps, idst, 1)
                         for g in range(NPB)]
                # (scalar_idx, dve_idx|None, group); groups don't mix in a
                # pair. The packed group (hm1/hp1p) runs FIRST so the hp1
                # unpack DMAs overlap the rest of the attention phase.
                prs = [(85 + 2 * i, 85 + 2 * i + 1, 1) for i in range(11)]
                prs += [(2 * i, 2 * i + 1, 0) for i in range(42)] + [(84, None, 0)]
                # unpaired pairs skip the DVE pair-add; the (underloaded) PE
                # consumes both tiles separately instead. DD pairs shift h
                # production from Scalar to DVE for engine balance.
                unpaired = {(i * 53) // 39 for i in range(39)}
                DD = set()
                n_consume = [0, 0]
                for pi, (ia, ib, gi) in enumerate(prs):
                    n_consume[gi] += 2 if (ib is not None and pi in unpaired) else 1
                seen = [0, 0]

                def feed_pe(s_t, psum, lhsT, gi):
                    first = seen[gi] == 0
                    seen[gi] += 1
                    last = seen[gi] == n_consume[gi]
                    nc.tensor.matmul(psum[:, 0:512], lhsT, s_t[:, 0:512],
                                     start=first, stop=last)
                    nc.tensor.matmul(psum[:, 512:MPAD], lhsT, s_t[:, 512:MPAD],
                                     start=first, stop=last)

                def emit_consume(pend):
                    a_t, b_t, psum, lhsT, gi, pi = pend
                    if b_t is None:
                        feed_pe(a_t, psum, lhsT, gi)
                    elif pi in unpaired:
                        feed_pe(a_t, psum, lhsT, gi)
                        feed_pe(b_t, psum, lhsT, gi)
                    else:
                        s_t = hpool.tile([128, MPAD], BF16, tag="h", name="hs")
                        nc.vector.tensor_tensor(out=s_t, in0=a_t, in1=b_t, op=ALU.add)
                        feed_pe(s_t, psum, lhsT, gi)

                SS = {(i * 53) // 8 + 2 for i in range(8)}  # scalar-scalar pairs
                from collections import deque
                pipe = deque()
                for pi, (ia, ib, gi) in enumerate(prs):
                    j_src, j_bias, j_acc, j_ps, j_lhs, _ = jobs[ia]
                    a_t = produce_scalar(j_src, j_bias, j_acc)
                    b_t = None
                    if ib is not None:
                        k_src, k_bias, k_acc, _, _, _ = jobs[ib]
                        if pi in SS:
                            b_t = produce_scalar(k_src, k_bias, k_acc)
                        else:
                            b_t = produce_dve(k_src, k_bias, k_acc)
                    pipe.append((a_t, b_t, j_ps, j_lhs, gi, pi))
                    if len(pipe) > 2:
                        emit_consume(pipe.popleft())
                while pipe:
                    emit_consume(pipe.popleft())

            # zero the pa-pad column so the cl matmuls read finite values
            nc.vector.memset(hp0[:, LP3:88].bitcast(F32), 0.0)
            # unpack hp1p -> hp1[d, 4g+j]
            hp1 = wp.tile([32, 88], F32R, name="hp1")
            hp1_g = hp1.rearrange("d (g f) -> d g f", f=4)
            for j in range(4):
                nc.sync.dma_start(
                    out=hp1_g[:, :, j],
                    in_=hp1p[32 * j:32 * j + 32, 0:NPB])

            # ---- peptide attention gate ----
            # catt[d,p] = sigmoid(sum_c hp[c,p]/LM3 * Wa[c,d] + ba[d]); 1/LM3 folded into wca
            cl0_ps = ps.tile([128, LP3E], F32, name="cl0_ps", tag="ps")
            nc.tensor.matmul(cl0_ps, wcaa[:, 0:128], hp0[:, 0:LP3E], start=True, stop=False)
            nc.tensor.matmul(cl0_ps, wcab[:, 0:128], hp1[:, 0:LP3E], start=False, stop=True)
            catt0 = wp.tile([128, LP3E], F32, name="catt0")
            nc.scalar.activation(out=catt0, in_=cl0_ps, func=AF.Sigmoid, bias=ba[:, 0:1])
            cl1_ps = ps.tile([32, LP3E], F32, name="cl1_ps", tag="ps")
            nc.tensor.matmul(cl1_ps, wcaa[:, 128:C4], hp0[:, 0:LP3E], start=True, stop=False)
            nc.tensor.matmul(cl1_ps, wcab[:, 128:C4], hp1[:, 0:LP3E], start=False, stop=True)
            catt1 = wp.tile([32, LP3E], F32, name="catt1")
            nc.scalar.activation(out=catt1, in_=cl1_ps, func=AF.Sigmoid, bias=ba[0:32, 1:2])

            pg0 = wp.tile([128, LP3E], F32, name="pg0")
            nc.vector.tensor_scalar(out=catt0, in0=catt0, scalar1=0.5, scalar2=None, op0=ALU.add)
            nc.vector.tensor_tensor(out=pg0, in0=catt0, in1=pc0, op=ALU.mult)
            pv0 = wp.tile([128, 1], F32R, name="pv0")
            nc.vector.tensor_reduce(out=pv0, in_=pg0[:, 0:LP3], op=ALU.max, axis=AX.X)
            pg1 = wp.tile([32, LP3E], F32, name="pg1")
            nc.vector.tensor_scalar(out=catt1, in0=catt1, scalar1=0.5, scalar2=None, op0=ALU.add)
            nc.vector.tensor_tensor(out=pg1, in0=catt1, in1=pc1, op=ALU.mult)
            pv1 = wp.tile([32, 1], F32R, name="pv1")
            nc.vector.tensor_reduce(out=pv1, in_=pg1[:, 0:LP3], op=ALU.max, axis=AX.X)

            # ---- MHC attention gate ----
            hm0 = wp.tile([128, LM3E], F32R, name="hm0")
            nc.scalar.copy(hm0, hm0_ps[:, 0:LM3E])
            hm1 = wp.tile([32, LM3E], F32R, name="hm1")
            nc.scalar.copy(hm1, hm1_ps[:, 0:LM3E])

            matt0 = wp.tile([128, LM3E], F32, name="matt0")
            ml0_ps = ps.tile([128, LM3E], F32, name="ml0_ps", tag="ps")
            for lo, hi in ((0, 512), (512, LM3E)):
                nc.tensor.matmul(ml0_ps[:, lo:hi], wmaa2[:, 0:128], hm0[:, lo:hi], start=True, stop=False)
                nc.tensor.matmul(ml0_ps[:, lo:hi], wmab2[:, 0:128], hm1[:, lo:hi], start=False, stop=True)
            nc.scalar.activation(out=matt0, in_=ml0_ps, func=AF.Sigmoid, bias=ba[:, 0:1])
            matt1 = wp.tile([32, LM3E], F32, name="matt1")
            ml1_ps = ps.tile([32, LM3E], F32, name="ml1_ps", tag="ps")
            for lo, hi in ((0, 512), (512, LM3E)):
                nc.tensor.matmul(ml1_ps[:, lo:hi], wmaa2[:, 128:C4], hm0[:, lo:hi], start=True, stop=False)
                nc.tensor.matmul(ml1_ps[:, lo:hi], wmab2[:, 128:C4], hm1[:, lo:hi], start=False, stop=True)
            nc.scalar.activation(out=matt1, in_=ml1_ps, func=AF.Sigmoid, bias=ba[0:32, 1:2])

            mg0 = wp.tile([128, LM3E], F32, name="mg0")
            nc.vector.tensor_scalar(out=matt0, in0=matt0, scalar1=0.5, scalar2=None, op0=ALU.add)
            nc.vector.tensor_tensor(out=mg0, in0=matt0, in1=mc0, op=ALU.mult)
            mv0 = wp.tile([128, 1], F32R, name="mv0")
            nc.vector.tensor_reduce(out=mv0, in_=mg0[:, 0:LM3], op=ALU.max, axis=AX.X)
            mg1 = wp.tile([32, LM3E], F32, name="mg1")
            nc.vector.tensor_scalar(out=matt1, in0=matt1, scalar1=0.5, scalar2=None, op0=ALU.add)
            nc.vector.tensor_tensor(out=mg1, in0=matt1, in1=mc1, op=ALU.mult)
            mv1 = wp.tile([32, 1], F32R, name="mv1")
            nc.vector.tensor_reduce(out=mv1, in_=mg1[:, 0:LM3], op=ALU.max, axis=AX.X)

            # ---- FC head: weights stream as the moving operand ----
            # f_psum rows [1, N]; fold back to [128, N/128] columns with
            # f[kp+a] -> fold[p, a] DMAs (W blocks host-permuted to match).
            def row_fc(name, pieces, ncols):
                """pieces: list of (lhsT [K,1], rhs [K, ncols]); psum [1, ncols]."""
                q = ps.tile([1, ncols], F32, name=name + "_ps", tag="ps")
                n = len(pieces)
                for i, (v, wt) in enumerate(pieces):
                    nc.tensor.matmul(q, v, wt, start=(i == 0), stop=(i == n - 1))
                r = wp.tile([1, ncols], F32, name=name + "_row")
                nc.scalar.copy(r, q)
                return r

            def bias_lrelu(name, z, bias, ncols):
                zb = wp.tile([128, ncols], F32, name=name + "_zb")
                nc.vector.tensor_tensor(out=zb, in0=z, in1=bias, op=ALU.add)
                zs = wp.tile([128, ncols], F32, name=name + "_zs")
                nc.vector.tensor_scalar(out=zs, in0=zb, scalar1=0.01, scalar2=None, op0=ALU.mult)
                fo = wp.tile([128, ncols], F32R, name=name)
                nc.vector.tensor_tensor(out=fo, in0=zb, in1=zs, op=ALU.max)
                return fo

            ones11 = wp.tile([1, 1], F32, name="ones11")
            nc.vector.memset(ones11, 1.0)

            def fold_rows(name, rows, ncols):
                """Transpose row tiles [1, 128*k] into [128, sum k] columns via
                PE outer products (lhsT row chunk x ones -> psum column)."""
                tz = ps.tile([128, ncols], F32, name=name + "_tz", tag="ps")
                a = 0
                for row in rows:
                    for c in range(row.shape[1] // 128):
                        nc.tensor.matmul(tz[:, a:a + 1], row[0:1, 128 * c:128 * (c + 1)],
                                         ones11, start=True, stop=True)
                        a += 1
                z = wp.tile([128, ncols], F32, name=name + "_z")
                nc.scalar.copy(z, tz)
                return z

            # f1: [320] @ W1[320,1024] -> two 512-col rows -> fold [128, 8]
            r1a = row_fc("f1a", [(pv0, w1r0[:, 0:512]), (pv1, w1r1[:, 0:512]),
                                 (mv0, w1r2[:, 0:512]), (mv1, w1r3[:, 0:512])], 512)
            r1b = row_fc("f1b", [(pv0, w1r0[:, 512:1024]), (pv1, w1r1[:, 512:1024]),
                                 (mv0, w1r2[:, 512:1024]), (mv1, w1r3[:, 512:1024])], 512)
            f1z = fold_rows("f1z", [r1a, r1b], 8)
            f1 = bias_lrelu("f1", f1z, b1, 8)

            # f2: f1[1024] @ W2[1024,1024]; W2 block a = rows [128a:128a+128]
            r2a = row_fc("f2a", [(f1[:, a:a + 1], w2[:, 1024 * a: 1024 * a + 512])
                                 for a in range(8)], 512)
            r2b = row_fc("f2b", [(f1[:, a:a + 1], w2[:, 1024 * a + 512: 1024 * (a + 1)])
                                 for a in range(8)], 512)
            f2z = fold_rows("f2z", [r2a, r2b], 8)
            f2 = bias_lrelu("f2", f2z, b2, 8)

            # f3: f2[1024] @ W3[1024,512]; W3 block a = rows [128a:128a+128]
            r3 = row_fc("f3r", [(f2[:, a:a + 1], w3[:, 512 * a: 512 * (a + 1)])
                                for a in range(8)], 512)
            f3z = fold_rows("f3z", [r3], 4)
            f3 = bias_lrelu("f3", f3z, b3, 4)

            # out: f3[512] @ Wo[512,2]; Wo block c = rows [128c:128c+128]
            qo = ps.tile([1, 2], F32, name="qo_ps", tag="ps")
            for c in range(4):
                nc.tensor.matmul(qo, f3[:, c:c + 1], wo[:, 2 * c:2 * c + 2],
                                 start=(c == 0), stop=(c == 3))
            o_sb = wp.tile([1, 2], F32, name="o_sb")
            nc.vector.tensor_tensor(out=o_sb, in0=qo, in1=bo, op=ALU.add)
            nc.sync.dma_start(out=out_e[:], in_=o_sb)

            if DEBUG:
                def dump(name, t, shape):
                    e = nc.declare_dram_parameter(name, list(shape), F32, isOutput=True)
                    nc.sync.dma_start(out=e[:], in_=t.bitcast(F32))
                dh0 = wp.tile([128, MPAD], BF16, name="dh0")
                dacc0 = wp.tile([128, 1], F32, name="dacc0")
                nc.scalar.activation(out=dh0, in_=ma0, func=AF.Relu,
                                     bias=pa0[:, 1:2], accum_out=dacc0[:, 0:1])
                dh1 = wp.tile([128, MPAD], BF16, name="dh1")
                dacc1 = wp.tile([128, 1], F32, name="dacc1")
                nc.vector.tensor_scalar(out=dh1, in0=ma0, scalar1=pa0[:, 1:2],
                                        scalar2=0.0, op0=ALU.add, op1=ALU.max)
                nc.vector.tensor_reduce(out=dacc1, in_=dh1, op=ALU.add, axis=AX.X)
                dh0f = wp.tile([128, MPAD], F32, name="dh0f")
                nc.scalar.copy(dh0f, dh0)
                dh1f = wp.tile([128, MPAD], F32, name="dh1f")
                nc.scalar.copy(dh1f, dh1)
                dump("d_h0", dh0f, [128, MPAD])
                dump("d_h1", dh1f, [128, MPAD])
                dump("d_acc0", dacc0, [128, 1])
                dump("d_acc1", dacc1, [128, 1])
                dump("d_pc0", pc0, [128, LP3E])
                dump("d_mc0", mc0, [128, LM3E])
                dump("d_pa0", pa0, [128, LP3E])
                dump("d_hp0", hp0, [128, 88])
                dump("d_hm0", hm0, [128, LM3E])
                dump("d_catt0", catt0, [128, LP3E])
                dump("d_pv0", pv0, [128, 1])
                dump("d_pv1", pv1, [32, 1])
                dump("d_mv0", mv0, [128, 1])
                dump("d_mv1", mv1, [32, 1])
                dump("d_r1a", r1a, [1, 512])
                dump("d_r1b", r1b, [1, 512])
                dump("d_f1z", f1z, [128, 8])
                dump("d_f1", f1, [128, 8])
                dump("d_f2", f2, [128, 8])
                dump("d_f3", f3, [128, 4])

    _split_excess_waits(nc, max_waits=1)
    return nc


_PROGRAM = None


def _get_program():
    global _PROGRAM
    if _PROGRAM is None:
        _PROGRAM = _build_program()
    return _PROGRAM


def _r32(a):
    """Round fp32 to the fp32r 19-bit-mantissa grid (low 13 bits zeroed)."""
    a = np.ascontiguousarray(np.asarray(a, dtype=np.float32))
    return (a.view(np.uint32) & np.uint32(0xFFFFE000)).view(np.float32)


def _prep_weights(inp):
    """Host-side layout prep shared by all cores."""
    f = lambda x: np.ascontiguousarray(np.asarray(x, dtype=np.float32))

    def convw(w):  # [co, ci, k] -> [ci, k*co]
        w = np.asarray(w, dtype=np.float32)
        ci = w.shape[1]
        return np.ascontiguousarray(w.transpose(1, 2, 0).reshape(ci, -1))

    def bias2(b):  # [160] -> [128, 2] (col 0 = [0:128], col 1 rows 0:32 = [128:160])
        b = np.asarray(b, dtype=np.float32)
        out = np.zeros((128, 2), np.float32)
        out[:, 0] = b[0:128]
        out[0:32, 1] = b[128:160]
        return out

    def blk(w, k):  # [I, J], I = 128k -> [128, k*J], block a = rows [128a:128a+128]
        w = np.asarray(w, dtype=np.float32)
        j = w.shape[1]
        return np.ascontiguousarray(w.reshape(k, 128, j).transpose(1, 0, 2).reshape(128, k * j))

    wa985 = np.asarray(inp['Wa'], np.float32) / float(LM3)
    wa85 = np.asarray(inp['Wa'], np.float32) / float(LP3)
    w1 = np.asarray(inp['W1'], np.float32)

    biases = np.zeros((128, 14), np.float32)
    biases[0:CONV, 0] = f(inp['pb1'])
    biases[0:C2, 1] = f(inp['pb2'])
    biases[:, 2:4] = bias2(inp['pb3'])
    biases[0:CONV, 4] = f(inp['mb1'])
    biases[0:C2, 5] = f(inp['mb2'])
    biases[:, 6:8] = bias2(inp['mb3'])
    biases[:, 8:10] = bias2(inp['bpa'])
    biases[:, 10:12] = bias2(inp['bma'])
    biases[:, 12:14] = bias2(inp['ba'])

    fcb = np.zeros((128, 20), np.float32)
    fcb[:, 0:8] = f(inp['b1']).reshape(8, 128).T
    fcb[:, 8:16] = f(inp['b2']).reshape(8, 128).T
    fcb[:, 16:20] = f(inp['b3']).reshape(4, 128).T

    def stack3(w):  # [co, ci, 6] -> [3*ci, 2*co]: tap-triple t, rows [w_{3t};w_{3t+1};w_{3t+2}]
        w = np.asarray(w, dtype=np.float32)
        cols = []
        for t in range(2):
            blkw = np.concatenate([w[:, :, 3 * t].T, w[:, :, 3 * t + 1].T,
                                   w[:, :, 3 * t + 2].T], axis=0)
            cols.append(blkw)
        return np.ascontiguousarray(np.concatenate(cols, axis=1))

    def stack1(w):  # [co, ci, 4] -> [2*ci, 2*co]: tap-pair p, rows [w_{2p}; w_{2p+1}]
        w = np.asarray(w, dtype=np.float32)
        co, ci, k = w.shape
        cols = []
        for p2 in range(2):
            blkw = np.concatenate([w[:, :, 2 * p2].T, w[:, :, 2 * p2 + 1].T], axis=0)
            cols.append(blkw)  # [2*ci, co]
        return np.ascontiguousarray(np.concatenate(cols, axis=1))

    cat = lambda a, b: np.ascontiguousarray(np.concatenate([a, b], axis=1))
    d = {
        'cw1': _r32(cat(stack1(inp['pw1']), stack1(inp['mw1']))),
        'cw2': _r32(cat(convw(inp['pw2']), convw(inp['mw2']))),
        'cw3': _r32(cat(convw(inp['pw3']), convw(inp['mw3']))),
        'biases': biases,
        'watt': _r32(np.concatenate([f(inp['Wpa'][0:128]), f(inp['Wma'][0:128]),
                                     wa985[0:128], wa85[0:128]], axis=1)),
        'wattb': _r32(np.concatenate([f(inp['Wpa'][128:160]), f(inp['Wma'][128:160]),
                                      wa985[128:160], wa85[128:160]], axis=1)),
        'ids': np.ascontiguousarray(np.concatenate(
            [np.eye(128, dtype=ml_dtypes.bfloat16),
             np.tile(np.eye(32, dtype=ml_dtypes.bfloat16), (4, 1))], axis=1)),
        'w1a': _r32(cat(w1[0:128], w1[160:288])),
        'w1b': _r32(cat(w1[128:160], w1[288:320])),
        'w2': _r32(blk(inp['W2'], 8)), 'w3': _r32(blk(inp['W3'], 8)),
        'wo': _r32(blk(inp['Wo'], 4)),
        'fcb': fcb,
        'bo': f(inp['bo']).reshape(1, 2),
    }
    return d


def _emb_stack(emb, idx, width):
    """Host-side embedding gather, transposed, zero-padded to `width`+1 and
    2-tap stacked: rows 0:64 = e[:, j], rows 64:128 = e[:, j+1]."""
    e = np.asarray(emb, np.float32)[np.asarray(idx).astype(np.int64)].T
    pad = np.zeros((DIM, width + 1), np.float32)
    pad[:, 0:e.shape[1]] = e
    st = np.concatenate([pad[:, 0:width], pad[:, 1:width + 1]], axis=0)
    return _r32(np.ascontiguousarray(st))


def kernel(**inputs):
    nc = _get_program()
    shared = _prep_weights(inputs)
    peptide = np.asarray(inputs['peptide'])
    mhc = np.asarray(inputs['MHC'])
    in_maps = []
    for b in range(B):
        m = dict(shared)
        m['pe_st'] = _emb_stack(inputs['pep_emb'], peptide[b], PEE)
        m['me_st'] = _emb_stack(inputs['MHC_emb'] if 'MHC_emb' in inputs else inputs['mhc_emb'], mhc[b], MEE)
        in_maps.append(m)
    res = run_bass_kernel_spmd(nc, in_maps, core_ids=list(range(B)))
    return np.stack([np.asarray(res.results[i]['out']).reshape(2) for i in range(B)]).astype(np.float32)
